# revision 57
# baseline (speedup 1.0000x reference)
"""Trainium2 Bass kernel for the GODEFunc graph-ODE message-passing module.

Math (per batch b):
    xa   = sum_k conv_w[k] * (adj[k] @ x[b]) + conv_b
    W    = (w * clip(d,0,1)) @ w.T
    out  = tanh(0.5*sigmoid(alpha) * xa - 2*x[b] + x[b] @ W + x0[b]*sigmoid(beta))

Sharding: rows (nodes) split across 8 cores; each core computes its
1024-row slice of the output for all batches.  No collectives needed.

Host-side marshaling (inside kernel(), before upload):
  - adj row-slice is pre-transposed to contraction-major, quantized to fp8e4
    with conv_w[k] folded into each K-slice's quantization scale, and
    swizzled to [group, partition, k, sub, ns] so one 1MB DMA per group
    loads both K channels with 16KB-contiguous partition lines.
  - x is pre-packed [128, mc, b, f] fp8 (the DoubleRow K-pair dim is a
    stride-0 broadcast on-chip); xT/x0 slices pre-laid-out for contiguous
    DMA lines.

Per-core kernel:
  - stream adjT fp8 tiles; the K-channel pair rides DoubleRow's interleave
    dim so the PE folds K inside each matmul (no on-chip combine):
        psum += at[:,0,s].T @ x + at[:,1,s].T @ x
  - xw = x @ (W - 2I) via small f32 matmuls from xT, issued after the
    first two adj groups so they never stall the main MM stream.
  - epilogue: out = tanh(psum*0.5*sig(alpha)/S + xw + x0*sig(beta) + bias).
"""

import sys

for _p in ("/opt/trn_rl_repo",):
    if _p not in sys.path:
        sys.path.insert(0, _p)

from contextlib import ExitStack

import ml_dtypes
import numpy as np

import concourse.bass as bass
import concourse.mybir as mybir
import concourse.tile as tile
from concourse import bacc
from concourse.bass_utils import run_bass_kernel_spmd

dt = mybir.dt
AF = mybir.ActivationFunctionType
ALU = mybir.AluOpType

B, N, F, K = 4, 8192, 64, 2
N_CORES = 8
P = 128
G_ROWS = 512  # contraction rows per adj DMA group


def build_kernel(n=N, n_cores=N_CORES, b=B, f=F, k_dim=K):
    ns = n // n_cores          # rows per core
    nt_cnt = ns // P           # row tiles per core
    mc_cnt = n // P            # contraction chunks
    ng = n // G_ROWS           # adj DMA groups along contraction dim
    sub_cnt = G_ROWS // P      # 128-chunks per group
    bf = b * f

    nc = bacc.Bacc(None, target_bir_lowering=False, debug=False)

    adjT = nc.dram_tensor(
        "adjT", [ng, P, k_dim, sub_cnt, ns], dt.float8e4, kind="ExternalInput"
    )
    x4d = nc.dram_tensor("x4", [P, mc_cnt, b, f], dt.float8e4, kind="ExternalInput")
    xT = nc.dram_tensor("xT", [b, f, ns], dt.float16, kind="ExternalInput")
    x0 = nc.dram_tensor("x0", [ns, b, f], dt.bfloat16, kind="ExternalInput")
    # packed constants: [ident(0:64) | wT(64:128) | d(128) | cb(129) |
    #                    sinv(130) | alpha(131:139) | beta(139:147)]
    npack = 2 * f + 3 + 2 * (ns // P)
    cpack = nc.dram_tensor("cpack", [P, npack], dt.float32, kind="ExternalInput")
    y = nc.dram_tensor("y", [ns, bf], dt.float32, kind="ExternalOutput")

    with tile.TileContext(nc) as tc, ExitStack() as ctx:
        const = ctx.enter_context(tc.tile_pool(name="const", bufs=1))
        xres = ctx.enter_context(tc.tile_pool(name="xres", bufs=1))
        adj_pool = ctx.enter_context(tc.tile_pool(name="adjp", bufs=5))
        work = ctx.enter_context(tc.tile_pool(name="work", bufs=2))
        outp = ctx.enter_context(tc.tile_pool(name="outp", bufs=6))
        keep = ctx.enter_context(tc.tile_pool(name="keep", bufs=1))
        psy = ctx.enter_context(tc.tile_pool(name="psy", bufs=1, space="PSUM"))
        pxw_pool = ctx.enter_context(tc.tile_pool(name="pxw", bufs=3, space="PSUM"))
        paux = ctx.enter_context(tc.tile_pool(name="paux", bufs=1, space="PSUM"))

        # ---------------- adj group DMAs (sync queue, nothing ahead of them) --
        a_tiles = []
        for g in range(ng):
            at = adj_pool.tile(
                [P, k_dim, sub_cnt, ns], dt.float8e4, tag="adj", name="adj_t"
            )
            nc.sync.dma_start(out=at[:], in_=adjT[g])
            a_tiles.append(at)

        # ---------------- constants / gates: ONE packed DMA ------------------
        ct = const.tile([P, npack], dt.float32, tag="cpack")
        nc.scalar.dma_start(out=ct[:], in_=cpack[:, :])
        ident_f = ct[0:f, 0:f]
        wT = ct[0:f, f : 2 * f]
        d_sb = ct[0:f, 2 * f : 2 * f + 1]
        cb_sb = ct[:, 2 * f + 1 : 2 * f + 2]
        sinv_sb = ct[:, 2 * f + 2 : 2 * f + 3]
        al_sb = ct[:, 2 * f + 3 : 2 * f + 3 + nt_cnt]
        be_sb = ct[:, 2 * f + 3 + nt_cnt : 2 * f + 3 + 2 * nt_cnt]

        # xT resident for the xw matmuls: [f, b, ns] on 64 partitions;
        # split per batch so no single transfer hogs the scalar queue
        xTt = xres.tile([f, b, ns], dt.float16, tag="xTt")
        for bb in range(b):
            nc.scalar.dma_start(out=xTt[:, bb, :], in_=xT[bb])

        # resident x in two halves: the first unlocks DR groups 0-7 early
        half_mc = mc_cnt // 2
        x4h = []
        for h in range(2):
            cs = slice(h * half_mc, (h + 1) * half_mc)
            xt = xres.tile([P, half_mc, b, f], dt.float8e4, tag=f"x4_{h}")
            nc.scalar.dma_start(out=xt[:], in_=x4d[:, cs, :, :])
            x4h.append(xt)


        # ---------------- W' = (w * clip(d,0,1)) @ w.T - 2I ----------------
        # issued on DVE BEFORE anything sigmoid-dependent: these ops gate the
        # fp32 xw matmuls sitting in the PE FIFO, and must not queue behind
        # the (late-scheduled) sigmoid table load.
        dc = const.tile([f, 1], dt.float32, tag="dc")
        nc.vector.tensor_scalar(dc[:], d_sb, 0.0, 1.0, ALU.max, ALU.min)
        wdc = const.tile([f, f], dt.float32, tag="wdc")
        nc.vector.tensor_scalar(wdc[:], wT, dc[:], None, ALU.mult)
        pw2 = paux.tile([f, f], dt.float32, tag="paux")
        nc.tensor.matmul(pw2[:], wT, wdc[:], start=True, stop=True)
        wp = const.tile([f, f], dt.float16, tag="wp")
        nc.vector.scalar_tensor_tensor(
            wp[:], ident_f, -2.0, pw2[:], ALU.mult, ALU.add
        )

        siga = const.tile([P, nt_cnt], dt.float32, tag="siga")
        nc.scalar.activation(siga[:], al_sb, AF.Sigmoid)
        sigb = const.tile([P, nt_cnt], dt.float32, tag="sigb")
        nc.scalar.activation(sigb[:], be_sb, AF.Sigmoid)
        # bias_cb[p, nt] = 0.5 * sigmoid(alpha) * conv_b
        bias_cb = const.tile([P, nt_cnt], dt.float32, tag="bias_cb")
        nc.vector.tensor_scalar(
            bias_cb[:], siga[:], cb_sb, 0.5, ALU.mult, ALU.mult
        )
        # sa[p, nt] = 0.5 * sigmoid(alpha) / S  (psum descale + alpha gate)
        sa = const.tile([P, nt_cnt], dt.float32, tag="sa")
        nc.vector.tensor_scalar(sa[:], siga[:], sinv_sb, None, ALU.mult)

        # ---------------- psum accumulators: two row-tiles per bank ----------
        n_banks = (nt_cnt + 1) // 2
        psum_y = [
            psy.tile([P, 2 * bf], dt.float32, tag=f"y{i}", name=f"psum_y{i}")
            for i in range(n_banks)
        ]

        def y_region(ntt):
            return psum_y[ntt // 2][:, (ntt % 2) * bf : (ntt % 2 + 1) * bf]

        def mm_one(g, s, ntt):
            mc = g * sub_cnt + s
            xt = x4h[mc // half_mc]
            nc.tensor.matmul(
                y_region(ntt),
                a_tiles[g][:, :, s, ntt * P : (ntt + 1) * P],
                xt[:, mc % half_mc, None, :, :].to_broadcast((P, k_dim, b, f)),
                start=(mc == 0),
                stop=(mc == mc_cnt - 1),
                skip_group_check=True,
                perf_mode=mybir.MatmulPerfMode.DoubleRow,
            )

        def mm_group(g, ntt_major=False):
            if ntt_major:
                # last group: each row-tile's accumulation finishes as early
                # as possible so the epilogue overlaps the remaining matmuls
                for ntt in range(nt_cnt):
                    for s in range(sub_cnt):
                        mm_one(g, s, ntt)
            else:
                for s in range(sub_cnt):
                    for ntt in range(nt_cnt):
                        mm_one(g, s, ntt)

        # groups 0-1 before the xw prologue so the PE never stalls on it
        mm_group(0)
        mm_group(1)

        # ---------------- xw = x @ (W - 2I), plus x0/beta epilogue prep ----
        x0t = xres.tile([P, nt_cnt, bf], dt.bfloat16, tag="x0t")
        nc.scalar.dma_start(
            out=x0t[:], in_=x0.rearrange("(t p) b f -> p t (b f)", p=P)
        )
        xwx0 = []
        for ntt in range(nt_cnt):
            rows = slice(ntt * P, (ntt + 1) * P)
            pxw = pxw_pool.tile([P, bf], dt.float32, tag="pxw")
            for bb in range(b):
                nc.tensor.matmul(
                    pxw[:, bb * f : (bb + 1) * f],
                    xTt[:, bb, rows],
                    wp[:],
                    start=True,
                    stop=True,
                )
            acc = keep.tile([P, bf], dt.float32, tag=f"xwx0_{ntt}")
            # acc = x0 * sigmoid(beta) + xw
            nc.vector.scalar_tensor_tensor(
                acc[:], x0t[:, ntt, :], sigb[:, ntt : ntt + 1], pxw[:],
                ALU.mult, ALU.add,
            )
            xwx0.append(acc)

        # ---------------- remaining adj groups --------------------------------
        for g in range(2, ng):
            mm_group(g, ntt_major=(g == ng - 1))

        # ---------------- epilogue: tanh(psum*sa + xwx0 + bias) ---------------
        for ntt in range(nt_cnt):
            rows = slice(ntt * P, (ntt + 1) * P)
            acc = outp.tile([P, bf], dt.float32, tag="eacc")
            nc.vector.scalar_tensor_tensor(
                acc[:], y_region(ntt), sa[:, ntt : ntt + 1], xwx0[ntt][:],
                ALU.mult, ALU.add,
            )
            outt = outp.tile([P, bf], dt.float32, tag="outt")
            nc.scalar.activation(
                outt[:], acc[:], AF.Tanh, bias=bias_cb[:, ntt : ntt + 1]
            )
            nc.sync.dma_start(out=y[rows, :], in_=outt[:])

    nc.finalize()
    return nc


_NC_CACHE = {}


def _get_nc(key=(N, N_CORES, B, F, K)):
    if key not in _NC_CACHE:
        _NC_CACHE[key] = build_kernel(*key)
    return _NC_CACHE[key]


def make_in_maps(x, x0, adj, alpha, beta, w, d, conv_w, conv_b, n_cores=N_CORES):
    """Marshal the full inputs into per-core shards (layout + dtype only)."""
    n = x.shape[1]
    b, f = x.shape[0], x.shape[2]
    ns = n // n_cores
    f32 = np.float32
    f8 = ml_dtypes.float8_e4m3

    # fp8 scale for adj: conv_w[k] is folded into each k-slice's scale, and
    # |adj * S * cw_k| must stay safely below the e4m3 max (240).
    amax = float(np.abs(adj).max())
    cwmax = max(float(np.abs(conv_w).max()), 1e-30)
    S = 2.0 ** np.floor(np.log2(180.0 / max(amax * cwmax, 1e-30)))
    sinv = np.array([0.5 / S], dtype=f32)

    # per-k quantization scale with conv_w folded in (contribution-weighted)
    kscale = (np.asarray(conv_w, dtype=f32) * f32(S)).reshape(-1, 1, 1)

    # moving operand: x chunked along contraction dim, all batches stacked;
    # the DoubleRow K-pair dim is a stride-0 broadcast on-chip, so no dup.
    x4 = np.ascontiguousarray(
        x.reshape(b, n // P, P, f).transpose(2, 1, 0, 3)
    ).astype(f8)  # [128, mc, b, f]

    nt_cnt = ns // P

    def cpack_core(c):
        """[ident | wT | d | cb | sinv | alpha | beta] packed as [128, npack]."""
        rows = slice(c * ns, (c + 1) * ns)
        npack = 2 * f + 3 + 2 * nt_cnt
        ct = np.zeros((P, npack), dtype=f32)
        ct[0:f, 0:f] = np.eye(f, dtype=f32)
        ct[0:f, f : 2 * f] = np.asarray(w, dtype=f32).T
        ct[0:f, 2 * f] = np.asarray(d, dtype=f32)
        ct[:, 2 * f + 1] = f32(np.asarray(conv_b).reshape(-1)[0])
        ct[:, 2 * f + 2] = sinv[0]
        ct[:, 2 * f + 3 : 2 * f + 3 + nt_cnt] = (
            np.asarray(alpha[rows], dtype=f32).reshape(nt_cnt, P).T
        )
        ct[:, 2 * f + 3 + nt_cnt : 2 * f + 3 + 2 * nt_cnt] = (
            np.asarray(beta[rows], dtype=f32).reshape(nt_cnt, P).T
        )
        return ct

    ng = n // G_ROWS
    sub = G_ROWS // P
    in_maps = []
    for c in range(n_cores):
        rows = slice(c * ns, (c + 1) * ns)
        adj_s = (adj[:, rows, :].astype(f32) * kscale).astype(f8)  # [K, ns, N]
        adjT_c = adj_s.transpose(0, 2, 1)                          # [K, N, ns]
        # swizzle to [ng, P, K, sub, ns]: partition p of group g holds both
        # K channels of the sub contraction chunks (g*G_ROWS + s*128 + p)
        adjT_c = np.ascontiguousarray(
            adjT_c.reshape(2, ng, sub, P, ns).transpose(1, 3, 0, 2, 4)
        )
        in_maps.append(
            {
                "adjT": adjT_c,
                "x4": x4,
                "xT": np.ascontiguousarray(
                    x[:, rows, :].transpose(0, 2, 1), dtype=np.float16
                ),
                "x0": np.ascontiguousarray(
                    x0[:, rows, :].transpose(1, 0, 2)
                ).astype(ml_dtypes.bfloat16),
                "cpack": cpack_core(c),
            }
        )
    return in_maps


def kernel(x, x0, adj, alpha, beta, w, d, conv_w, conv_b):
    x = np.asarray(x)
    x0 = np.asarray(x0)
    adj = np.asarray(adj)
    alpha = np.asarray(alpha)
    beta = np.asarray(beta)
    w = np.asarray(w)
    d = np.asarray(d)
    conv_w = np.asarray(conv_w)
    conv_b = np.asarray(conv_b)

    ns = N // N_CORES
    nc = _get_nc()
    in_maps = make_in_maps(x, x0, adj, alpha, beta, w, d, conv_w, conv_b)
    res = run_bass_kernel_spmd(nc, in_maps, core_ids=list(range(N_CORES)))
    out = np.concatenate(
        [res.results[c]["y"].reshape(ns, B, F).transpose(1, 0, 2) for c in range(N_CORES)],
        axis=1,
    )
    return out.astype(np.float32)
